# revision 12
# baseline (speedup 1.0000x reference)
"""Trainium2 Bass kernel for nn_NewAttentionBlock (sparse_attention).

Joint softmax attention over a large masked "prior" KV block (S=4096) plus a
small "active" KV block (S=16), for B=8, H=16, Q=16, D=256, fp32.

Sharding: heads are split across the 8 NeuronCores (2 heads/core, tensor
parallel, no cross-core communication).  Each core processes its 16 (b,h)
pairs fully independently.

Per-(b,h) dataflow on a core (all matmuls in float32r on the PE):
  - K_prior/V_prior stream in as two 2 MiB slices each on the two HWDGE
    rings (K on SP, V on ACT), laid out so every SBUF partition receives
    one fully contiguous 16 KiB run from DRAM (s-row q*16+n lands on
    partition q).  This permutes s within each 128-row transpose tile,
    which is harmless: the same permutation is applied to the score
    columns (via the K^T transpose) and the V rows (same DMA layout),
    and softmax is permutation-invariant.  Small per-pair loads (Q,
    K_active, V_active) and the output store ride the gpsimd SWDGE ring
    so they never stall the two K/V streams.
  - Q, K_active are transposed on the PE (via identity matmul, identity
    DMA-loaded once as both f32 and f32r) to put D onto partitions.
  - Each 128-row s-tile of K is PE-transposed in f32r (1.5 cyc/row) into
    K^T chunks [128(d), 512(s)] in PSUM, then copied to SBUF by the
    Vector engine (kept off the ACT queue so exp never delays them).
  - scores[16, 512] chunks accumulate in PSUM (2 matmuls over the two
    128-halves of D), then ScalarE applies exp(SCALE*s) writing E to SBUF
    while accumulating the per-row sum (softmax denominator) for free.
  - E chunks are PE-transposed to P^T [s, q] and used as the stationary
    operand of the PV matmul against V tiles in natural [s, d] layout,
    accumulating attn_raw[16, 256] in PSUM across all 32 s-tiles + active.
  - The output is attn_raw * (1/denom) via a per-partition tensor_scalar.
For the loop-amplified timing path, tc.For_i(staggered_reset=True) avoids
the all-engine barrier at each iteration boundary so back-to-back
invocations pipeline.
The softmax max-subtraction is skipped: scaled scores are ~N(0,1) here so
exp() cannot overflow, and the result is mathematically identical.
prior_mask is all-ones per the problem spec; a numpy fallback handles the
(never expected) general case.
"""

import numpy as np

import concourse.bacc as bacc
import concourse.mybir as mybir
import concourse.tile as tile
from concourse.bass_utils import run_bass_kernel_spmd

B, H, QL, SP, D = 8, 16, 16, 4096, 256
SCALE = float(D) ** -0.5
N_CORES = 8
HPC = H // N_CORES          # heads per core
NP = B * HPC                # (b,h) pairs per core = 16
CHUNK = 512                 # score-chunk (columns per PSUM score tile)
NCH = SP // CHUNK           # 8 chunks / pair
TPC = CHUNK // 128          # 4 s-tiles per chunk
SLICE = 2048                # rows per K/V DMA (2 MiB, 16 KiB/partition)
RPP = SLICE // 128          # 16 s-rows per partition per slice
NSL = SP // SLICE           # 2 slices per pair per tensor

F32 = mybir.dt.float32
F32R = mybir.dt.float32r
EXP = mybir.ActivationFunctionType.Exp

_compiled = None


def _build(loop_n=None):
    nc = bacc.Bacc(
        "TRN2",
        target_bir_lowering=False,
        debug=False,
        num_devices=N_CORES,
    )
    q_d = nc.dram_tensor("q", [NP, QL, D], F32, kind="ExternalInput").ap()
    kp_d = nc.dram_tensor("kp", [NP, SP, D], F32, kind="ExternalInput").ap()
    vp_d = nc.dram_tensor("vp", [NP, SP, D], F32, kind="ExternalInput").ap()
    ka_d = nc.dram_tensor("ka", [NP, QL, D], F32, kind="ExternalInput").ap()
    va_d = nc.dram_tensor("va", [NP, QL, D], F32, kind="ExternalInput").ap()
    id_d = nc.dram_tensor("ident", [128, 128], F32, kind="ExternalInput").ap()
    out_d = nc.dram_tensor("out", [NP, QL, D], F32, kind="ExternalOutput").ap()

    with tile.TileContext(nc) as tc:
        with (
            tc.tile_pool(name="const", bufs=2) as constp,
            tc.tile_pool(name="kraw", bufs=4) as krawp,
            tc.tile_pool(name="vraw", bufs=4) as vrawp,
            tc.tile_pool(name="ktsb", bufs=8) as ktsbp,
            tc.tile_pool(name="esb", bufs=6) as esbp,
            tc.tile_pool(name="ptsb", bufs=4) as ptsbp,
            tc.tile_pool(name="small", bufs=6) as smallp,
            tc.tile_pool(name="qt", bufs=3) as qtp,
            tc.tile_pool(name="stat", bufs=3) as statp,
            tc.tile_pool(name="osb", bufs=3) as osbp,
            tc.tile_pool(name="ps_kt", bufs=4, space="PSUM") as ps_kt,
            tc.tile_pool(name="ps_s", bufs=2, space="PSUM") as ps_s,
            tc.tile_pool(name="ps_pt", bufs=1, space="PSUM") as ps_pt,
            tc.tile_pool(name="ps_pv", bufs=1, space="PSUM") as ps_pv,
        ):
            ident = constp.tile([128, 128], F32, tag="idf")
            nc.sync.dma_start(out=ident, in_=id_d)
            identr = constp.tile([128, 128], F32R, tag="idr")
            nc.sync.dma_start(out=identr, in_=id_d.bitcast(F32R))

            import contextlib
            loop_cm = (tc.For_i(0, loop_n, 1, staggered_reset=True)
                       if loop_n is not None else contextlib.nullcontext())
            with loop_cm:
              for p in range(NP):
                  # ---- small loads ----------------------------------------
                  q_sb = smallp.tile([QL, D], F32, tag="q")
                  nc.gpsimd.dma_start(out=q_sb, in_=q_d[p])
                  ka_sb = smallp.tile([QL, D], F32, tag="ka")
                  nc.gpsimd.dma_start(out=ka_sb, in_=ka_d[p])
                  va_sb = smallp.tile([QL, D], F32R, tag="va")
                  nc.gpsimd.dma_start(out=va_sb, in_=va_d[p].bitcast(F32R))

                  # ---- Q^T / K_active^T  [128, 2*16] ----------------------
                  qt_ps = ps_s.tile([128, 2 * QL], F32, tag="s")
                  kat_ps = ps_s.tile([128, 2 * QL], F32, tag="s")
                  for h in range(2):
                      nc.tensor.transpose(
                          qt_ps[:, h * QL:(h + 1) * QL],
                          q_sb[:, h * 128:(h + 1) * 128],
                          ident[:QL, :QL],
                      )
                      nc.tensor.transpose(
                          kat_ps[:, h * QL:(h + 1) * QL],
                          ka_sb[:, h * 128:(h + 1) * 128],
                          ident[:QL, :QL],
                      )
                  qt_sb = qtp.tile([128, 2 * QL], F32R, tag="qt")
                  nc.any.tensor_copy(qt_sb, qt_ps)
                  kat_sb = qtp.tile([128, 2 * QL], F32R, tag="kat")
                  nc.any.tensor_copy(kat_sb, kat_ps)

                  # ---- active scores + exp + P_active^T -------------------
                  dsum = statp.tile([QL, NCH + 1], F32, tag="dsum")
                  sa_ps = ps_s.tile([QL, QL], F32, tag="s")
                  nc.tensor.matmul(
                      sa_ps, qt_sb[:, 0:QL], kat_sb[:, 0:QL],
                      start=True, stop=False,
                  )
                  nc.tensor.matmul(
                      sa_ps, qt_sb[:, QL:2 * QL], kat_sb[:, QL:2 * QL],
                      start=False, stop=True,
                  )
                  ea_sb = esbp.tile([QL, QL], F32, tag="ea")
                  nc.scalar.activation(
                      ea_sb, sa_ps, EXP, scale=SCALE,
                      accum_out=dsum[:, NCH:NCH + 1],
                  )
                  pta_ps = ps_s.tile([QL, QL], F32, tag="s")
                  nc.tensor.transpose(pta_ps, ea_sb, ident[:QL, :QL])
                  pta_sb = qtp.tile([QL, QL], F32R, tag="pta")
                  nc.any.tensor_copy(pta_sb, pta_ps)

                  # ---- K/V prior streaming loads (2 MiB, 16K/partition) ---
                  kts, vts = [], []
                  for k in range(NSL):
                      kt = krawp.tile([128, RPP, D], F32R, tag="kraw")
                      nc.sync.dma_start(
                          out=kt,
                          in_=kp_d[p, k * SLICE:(k + 1) * SLICE, :].rearrange(
                              "(q n) d -> q n d", q=128).bitcast(F32R),
                      )
                      kts.append(kt)
                  for k in range(NSL):
                      vt = vrawp.tile([128, RPP, D], F32R, tag="vraw")
                      nc.scalar.dma_start(
                          out=vt,
                          in_=vp_d[p, k * SLICE:(k + 1) * SLICE, :].rearrange(
                              "(q n) d -> q n d", q=128).bitcast(F32R),
                      )
                      vts.append(vt)

                  # ---- prior chunks ---------------------------------------
                  pt_ps = ps_pt.tile([128, NCH * TPC * QL], F32, tag="pt")
                  pv_ps = ps_pv.tile([QL, D], F32, tag="pv")
                  for c in range(NCH):
                      ktp0 = ps_kt.tile([128, CHUNK], F32R, tag="kt")
                      ktp1 = ps_kt.tile([128, CHUNK], F32R, tag="kt")
                      for j in range(TPC):
                          t = c * TPC + j
                          kt_raw = kts[t // RPP]
                          col = t % RPP
                          nc.tensor.transpose(
                              ktp0[:, j * 128:(j + 1) * 128],
                              kt_raw[:, col, 0:128], identr)
                          nc.tensor.transpose(
                              ktp1[:, j * 128:(j + 1) * 128],
                              kt_raw[:, col, 128:256], identr)
                      kt0 = ktsbp.tile([128, CHUNK], F32R, tag="kt0")
                      kt1 = ktsbp.tile([128, CHUNK], F32R, tag="kt1")
                      nc.vector.tensor_copy(kt0, ktp0)
                      nc.vector.tensor_copy(kt1, ktp1)

                      s_ps = ps_s.tile([QL, CHUNK], F32, tag="s")
                      nc.tensor.matmul(
                          s_ps, qt_sb[:, 0:QL], kt0,
                          start=True, stop=False)
                      nc.tensor.matmul(
                          s_ps, qt_sb[:, QL:2 * QL], kt1,
                          start=False, stop=True)

                      e_sb = esbp.tile([QL, CHUNK], F32, tag="e")
                      nc.scalar.activation(
                          e_sb, s_ps, EXP, scale=SCALE,
                          accum_out=dsum[:, c:c + 1],
                      )
                      for j in range(TPC):
                          nc.tensor.transpose(
                              pt_ps[:, c * TPC * QL + j * QL:
                                    c * TPC * QL + (j + 1) * QL],
                              e_sb[:, j * 128:(j + 1) * 128],
                              ident[:QL, :QL],
                          )
                      ptc = ptsbp.tile([128, TPC * QL], F32R, tag="ptc")
                      nc.any.tensor_copy(
                          ptc, pt_ps[:, c * TPC * QL:(c + 1) * TPC * QL])
                      for j in range(TPC):
                          t = c * TPC + j
                          v_raw = vts[t // RPP]
                          nc.tensor.matmul(
                              pv_ps,
                              ptc[:, j * QL:(j + 1) * QL],
                              v_raw[:, t % RPP, :],
                              start=(t == 0), stop=False,
                          )
                  # active PV contribution last (closes the accumulation)
                  nc.tensor.matmul(
                      pv_ps, pta_sb, va_sb, start=False, stop=True)

                  # ---- normalize + store ----------------------------------
                  den = statp.tile([QL, 1], F32, tag="den")
                  nc.vector.reduce_sum(
                      out=den, in_=dsum[:, 0:NCH + 1], axis=mybir.AxisListType.X)
                  rec = statp.tile([QL, 1], F32, tag="rec")
                  nc.vector.reciprocal(rec, den)
                  o_sb = osbp.tile([QL, D], F32, tag="o")
                  nc.vector.tensor_scalar_mul(o_sb, pv_ps, rec)
                  nc.gpsimd.dma_start(out=out_d[p], in_=o_sb)

    nc.compile()
    return nc


def _get_compiled():
    global _compiled
    if _compiled is None:
        _compiled = _build()
    return _compiled


def make_in_maps(Q, K_prior, V_prior, K_active, V_active):
    in_maps = []
    for c in range(N_CORES):
        hs = slice(c * HPC, (c + 1) * HPC)
        in_maps.append({
            "q": np.ascontiguousarray(Q[:, hs]).reshape(NP, QL, D),
            "kp": np.ascontiguousarray(K_prior[:, hs]).reshape(NP, SP, D),
            "vp": np.ascontiguousarray(V_prior[:, hs]).reshape(NP, SP, D),
            "ka": np.ascontiguousarray(K_active[:, hs]).reshape(NP, QL, D),
            "va": np.ascontiguousarray(V_active[:, hs]).reshape(NP, QL, D),
            "ident": np.eye(128, dtype=np.float32),
        })
    return in_maps


def gather_out(per_core_outs):
    full = np.stack(per_core_outs, axis=0).reshape(N_CORES, B, HPC, QL, D)
    return np.ascontiguousarray(
        full.transpose(1, 0, 2, 3, 4).reshape(B, H, QL, D))


def _numpy_fallback(Q, K_prior, V_prior, K_active, V_active, prior_mask):
    ps = np.einsum("bhqd,bhkd->bhqk", Q, K_prior) * SCALE
    as_ = np.einsum("bhqd,bhkd->bhqk", Q, K_active) * SCALE
    neg = np.finfo(np.float32).min
    ps = np.where(prior_mask, ps, neg)
    m = np.maximum(ps.max(-1, keepdims=True), as_.max(-1, keepdims=True))
    ep = np.exp(ps - m)
    ea = np.exp(as_ - m)
    den = ep.sum(-1, keepdims=True) + ea.sum(-1, keepdims=True)
    return (np.einsum("bhqk,bhkd->bhqd", (ep / den).astype(np.float32), V_prior)
            + np.einsum("bhqk,bhkd->bhqd", (ea / den).astype(np.float32),
                        V_active)).astype(np.float32)


def kernel(**inputs):
    Q = np.asarray(inputs["Q"], dtype=np.float32)
    K_prior = np.asarray(inputs["K_prior"], dtype=np.float32)
    V_prior = np.asarray(inputs["V_prior"], dtype=np.float32)
    K_active = np.asarray(inputs["K_active"], dtype=np.float32)
    V_active = np.asarray(inputs["V_active"], dtype=np.float32)
    prior_mask = np.asarray(inputs["prior_mask"])

    if not prior_mask.all():
        # Spec guarantees an all-ones mask; general masks take the slow path.
        return _numpy_fallback(Q, K_prior, V_prior, K_active, V_active,
                               prior_mask)

    nc = _get_compiled()
    res = run_bass_kernel_spmd(
        nc,
        make_in_maps(Q, K_prior, V_prior, K_active, V_active),
        core_ids=list(range(N_CORES)),
    )
    return gather_out([res.results[c]["out"] for c in range(N_CORES)])


# revision 13
# speedup vs baseline: 1.0712x; 1.0712x over previous
"""Trainium2 Bass kernel for nn_NewAttentionBlock (sparse_attention).

Joint softmax attention over a large masked "prior" KV block (S=4096) plus a
small "active" KV block (S=16), for B=8, H=16, Q=16, D=256, fp32.

Sharding: heads are split across the 8 NeuronCores (2 heads/core, tensor
parallel, no cross-core communication).  Each core processes its 16 (b,h)
pairs fully independently.

Per-(b,h) dataflow on a core (all matmuls in float32r on the PE):
  - K_prior/V_prior stream in as two 2 MiB slices each on the two HWDGE
    rings (K on SP, V on ACT), laid out so every SBUF partition receives
    one fully contiguous 16 KiB run from DRAM (s-row q*16+n lands on
    partition q).  This permutes s within each 128-row transpose tile,
    which is harmless: the same permutation is applied to the score
    columns (via the K^T transpose) and the V rows (same DMA layout),
    and softmax is permutation-invariant.  Small per-pair loads (Q,
    K_active, V_active) and the output store ride the gpsimd SWDGE ring
    so they never stall the two K/V streams.
  - Q, K_active are transposed on the PE (via identity matmul, identity
    DMA-loaded once as both f32 and f32r) to put D onto partitions.
  - Each 128-row s-tile of K is PE-transposed in f32r (1.5 cyc/row) into
    K^T chunks [128(d), 512(s)] in PSUM, then copied to SBUF by the
    Vector engine (kept off the ACT queue so exp never delays them).
  - scores[16, 512] chunks accumulate in PSUM (2 matmuls over the two
    128-halves of D), then ScalarE applies exp(SCALE*s) writing E to SBUF
    while accumulating the per-row sum (softmax denominator) for free.
  - E chunks are PE-transposed to P^T [s, q] and used as the stationary
    operand of the PV matmul against V tiles in natural [s, d] layout,
    accumulating attn_raw[16, 256] in PSUM across all 32 s-tiles + active.
  - The output is attn_raw * (1/denom) via a per-partition tensor_scalar.
For the loop-amplified timing path, tc.For_i(staggered_reset=True) avoids
the all-engine barrier at each iteration boundary so back-to-back
invocations pipeline.
The softmax max-subtraction is skipped: scaled scores are ~N(0,1) here so
exp() cannot overflow, and the result is mathematically identical.
prior_mask is all-ones per the problem spec; a numpy fallback handles the
(never expected) general case.
"""

import numpy as np

import concourse.bacc as bacc
import concourse.mybir as mybir
import concourse.tile as tile
from concourse.bass_utils import run_bass_kernel_spmd

B, H, QL, SP, D = 8, 16, 16, 4096, 256
SCALE = float(D) ** -0.5
N_CORES = 8
HPC = H // N_CORES          # heads per core
NP = B * HPC                # (b,h) pairs per core = 16
CHUNK = 512                 # score-chunk (columns per PSUM score tile)
NCH = SP // CHUNK           # 8 chunks / pair
TPC = CHUNK // 128          # 4 s-tiles per chunk
SLICE = 2048                # rows per K/V DMA (2 MiB, 16 KiB/partition)
RPP = SLICE // 128          # 16 s-rows per partition per slice
NSL = SP // SLICE           # 2 slices per pair per tensor

F32 = mybir.dt.float32
F32R = mybir.dt.float32r
EXP = mybir.ActivationFunctionType.Exp

_compiled = None


def _build(loop_n=None):
    nc = bacc.Bacc(
        "TRN2",
        target_bir_lowering=False,
        debug=False,
        num_devices=N_CORES,
    )
    q_d = nc.dram_tensor("q", [NP, QL, D], F32, kind="ExternalInput").ap()
    kp_d = nc.dram_tensor("kp", [NP, SP, D], F32, kind="ExternalInput").ap()
    vp_d = nc.dram_tensor("vp", [NP, SP, D], F32, kind="ExternalInput").ap()
    ka_d = nc.dram_tensor("ka", [NP, QL, D], F32, kind="ExternalInput").ap()
    va_d = nc.dram_tensor("va", [NP, QL, D], F32, kind="ExternalInput").ap()
    id_d = nc.dram_tensor("ident", [128, 128], F32, kind="ExternalInput").ap()
    out_d = nc.dram_tensor("out", [NP, QL, D], F32, kind="ExternalOutput").ap()

    with tile.TileContext(nc) as tc:
        with (
            tc.tile_pool(name="const", bufs=2) as constp,
            tc.tile_pool(name="kraw", bufs=4) as krawp,
            tc.tile_pool(name="vraw", bufs=4) as vrawp,
            tc.tile_pool(name="ktsb", bufs=6) as ktsbp,
            tc.tile_pool(name="esb", bufs=4) as esbp,
            tc.tile_pool(name="ptsb", bufs=4) as ptsbp,
            tc.tile_pool(name="small", bufs=6) as smallp,
            tc.tile_pool(name="qt", bufs=3) as qtp,
            tc.tile_pool(name="stat", bufs=3) as statp,
            tc.tile_pool(name="osb", bufs=3) as osbp,
            tc.tile_pool(name="ps_kt", bufs=4, space="PSUM") as ps_kt,
            tc.tile_pool(name="ps_s", bufs=2, space="PSUM") as ps_s,
            tc.tile_pool(name="ps_pt", bufs=1, space="PSUM") as ps_pt,
            tc.tile_pool(name="ps_pv", bufs=1, space="PSUM") as ps_pv,
        ):
            ident = constp.tile([128, 128], F32, tag="idf")
            nc.sync.dma_start(out=ident, in_=id_d)
            identr = constp.tile([128, 128], F32R, tag="idr")
            nc.sync.dma_start(out=identr, in_=id_d.bitcast(F32R))

            import contextlib
            loop_cm = (tc.For_i(0, loop_n, 1, staggered_reset=True)
                       if loop_n is not None else contextlib.nullcontext())
            with loop_cm:
              for p in range(NP):
                  # ---- small loads ----------------------------------------
                  q_sb = smallp.tile([QL, D], F32, tag="q")
                  nc.gpsimd.dma_start(out=q_sb, in_=q_d[p])
                  ka_sb = smallp.tile([QL, D], F32, tag="ka")
                  nc.gpsimd.dma_start(out=ka_sb, in_=ka_d[p])
                  va_sb = smallp.tile([QL, D], F32R, tag="va")
                  nc.gpsimd.dma_start(out=va_sb, in_=va_d[p].bitcast(F32R))

                  # ---- Q^T / K_active^T  [128, 2*16] ----------------------
                  qt_ps = ps_s.tile([128, 2 * QL], F32, tag="s")
                  kat_ps = ps_s.tile([128, 2 * QL], F32, tag="s")
                  for h in range(2):
                      nc.tensor.transpose(
                          qt_ps[:, h * QL:(h + 1) * QL],
                          q_sb[:, h * 128:(h + 1) * 128],
                          ident[:QL, :QL],
                      )
                      nc.tensor.transpose(
                          kat_ps[:, h * QL:(h + 1) * QL],
                          ka_sb[:, h * 128:(h + 1) * 128],
                          ident[:QL, :QL],
                      )
                  qt_sb = qtp.tile([128, 2 * QL], F32R, tag="qt")
                  nc.any.tensor_copy(qt_sb, qt_ps)
                  kat_sb = qtp.tile([128, 2 * QL], F32R, tag="kat")
                  nc.any.tensor_copy(kat_sb, kat_ps)

                  # ---- active scores + exp + P_active^T -------------------
                  dsum = statp.tile([QL, NCH + 1], F32, tag="dsum")
                  sa_ps = ps_s.tile([QL, QL], F32, tag="s")
                  nc.tensor.matmul(
                      sa_ps, qt_sb[:, 0:QL], kat_sb[:, 0:QL],
                      start=True, stop=False,
                  )
                  nc.tensor.matmul(
                      sa_ps, qt_sb[:, QL:2 * QL], kat_sb[:, QL:2 * QL],
                      start=False, stop=True,
                  )
                  ea_sb = esbp.tile([QL, QL], F32, tag="ea")
                  nc.scalar.activation(
                      ea_sb, sa_ps, EXP, scale=SCALE,
                      accum_out=dsum[:, NCH:NCH + 1],
                  )
                  pta_ps = ps_s.tile([QL, QL], F32, tag="s")
                  nc.tensor.transpose(pta_ps, ea_sb, ident[:QL, :QL])
                  pta_sb = qtp.tile([QL, QL], F32R, tag="pta")
                  nc.any.tensor_copy(pta_sb, pta_ps)

                  # ---- K/V prior streaming loads (2 MiB, 16K/partition) ---
                  kts, vts = [], []
                  for k in range(NSL):
                      kt = krawp.tile([128, RPP, D], F32R, tag="kraw")
                      nc.sync.dma_start(
                          out=kt,
                          in_=kp_d[p, k * SLICE:(k + 1) * SLICE, :].rearrange(
                              "(q n) d -> q n d", q=128).bitcast(F32R),
                      )
                      kts.append(kt)
                  for k in range(NSL):
                      vt = vrawp.tile([128, RPP, D], F32R, tag="vraw")
                      nc.scalar.dma_start(
                          out=vt,
                          in_=vp_d[p, k * SLICE:(k + 1) * SLICE, :].rearrange(
                              "(q n) d -> q n d", q=128).bitcast(F32R),
                      )
                      vts.append(vt)

                  # ---- prior chunks ---------------------------------------
                  pt_ps = ps_pt.tile([128, NCH * TPC * QL], F32, tag="pt")
                  pv_ps = ps_pv.tile([QL, D], F32, tag="pv")
                  for c in range(NCH):
                      ktp0 = ps_kt.tile([128, CHUNK], F32R, tag="kt")
                      ktp1 = ps_kt.tile([128, CHUNK], F32R, tag="kt")
                      for j in range(TPC):
                          t = c * TPC + j
                          kt_raw = kts[t // RPP]
                          col = t % RPP
                          nc.tensor.transpose(
                              ktp0[:, j * 128:(j + 1) * 128],
                              kt_raw[:, col, 0:128], identr)
                          nc.tensor.transpose(
                              ktp1[:, j * 128:(j + 1) * 128],
                              kt_raw[:, col, 128:256], identr)
                      kt0 = ktsbp.tile([128, CHUNK], F32R, tag="kt0")
                      kt1 = ktsbp.tile([128, CHUNK], F32R, tag="kt1")
                      nc.vector.tensor_copy(kt0, ktp0)
                      nc.scalar.copy(kt1, ktp1)

                      s_ps = ps_s.tile([QL, CHUNK], F32, tag="s")
                      nc.tensor.matmul(
                          s_ps, qt_sb[:, 0:QL], kt0,
                          start=True, stop=False)
                      nc.tensor.matmul(
                          s_ps, qt_sb[:, QL:2 * QL], kt1,
                          start=False, stop=True)

                      e_sb = esbp.tile([QL, CHUNK], F32, tag="e")
                      nc.scalar.activation(
                          e_sb, s_ps, EXP, scale=SCALE,
                          accum_out=dsum[:, c:c + 1],
                      )
                      for j in range(TPC):
                          nc.tensor.transpose(
                              pt_ps[:, c * TPC * QL + j * QL:
                                    c * TPC * QL + (j + 1) * QL],
                              e_sb[:, j * 128:(j + 1) * 128],
                              ident[:QL, :QL],
                          )
                      ptc = ptsbp.tile([128, TPC * QL], F32R, tag="ptc")
                      nc.any.tensor_copy(
                          ptc, pt_ps[:, c * TPC * QL:(c + 1) * TPC * QL])
                      for j in range(TPC):
                          t = c * TPC + j
                          v_raw = vts[t // RPP]
                          nc.tensor.matmul(
                              pv_ps,
                              ptc[:, j * QL:(j + 1) * QL],
                              v_raw[:, t % RPP, :],
                              start=(t == 0), stop=False,
                          )
                  # active PV contribution last (closes the accumulation)
                  nc.tensor.matmul(
                      pv_ps, pta_sb, va_sb, start=False, stop=True)

                  # ---- normalize + store ----------------------------------
                  den = statp.tile([QL, 1], F32, tag="den")
                  nc.vector.reduce_sum(
                      out=den, in_=dsum[:, 0:NCH + 1], axis=mybir.AxisListType.X)
                  rec = statp.tile([QL, 1], F32, tag="rec")
                  nc.vector.reciprocal(rec, den)
                  o_sb = osbp.tile([QL, D], F32, tag="o")
                  nc.vector.tensor_scalar_mul(o_sb, pv_ps, rec)
                  nc.gpsimd.dma_start(out=out_d[p], in_=o_sb)

    nc.compile()
    return nc


def _get_compiled():
    global _compiled
    if _compiled is None:
        _compiled = _build()
    return _compiled


def make_in_maps(Q, K_prior, V_prior, K_active, V_active):
    in_maps = []
    for c in range(N_CORES):
        hs = slice(c * HPC, (c + 1) * HPC)
        in_maps.append({
            "q": np.ascontiguousarray(Q[:, hs]).reshape(NP, QL, D),
            "kp": np.ascontiguousarray(K_prior[:, hs]).reshape(NP, SP, D),
            "vp": np.ascontiguousarray(V_prior[:, hs]).reshape(NP, SP, D),
            "ka": np.ascontiguousarray(K_active[:, hs]).reshape(NP, QL, D),
            "va": np.ascontiguousarray(V_active[:, hs]).reshape(NP, QL, D),
            "ident": np.eye(128, dtype=np.float32),
        })
    return in_maps


def gather_out(per_core_outs):
    full = np.stack(per_core_outs, axis=0).reshape(N_CORES, B, HPC, QL, D)
    return np.ascontiguousarray(
        full.transpose(1, 0, 2, 3, 4).reshape(B, H, QL, D))


def _numpy_fallback(Q, K_prior, V_prior, K_active, V_active, prior_mask):
    ps = np.einsum("bhqd,bhkd->bhqk", Q, K_prior) * SCALE
    as_ = np.einsum("bhqd,bhkd->bhqk", Q, K_active) * SCALE
    neg = np.finfo(np.float32).min
    ps = np.where(prior_mask, ps, neg)
    m = np.maximum(ps.max(-1, keepdims=True), as_.max(-1, keepdims=True))
    ep = np.exp(ps - m)
    ea = np.exp(as_ - m)
    den = ep.sum(-1, keepdims=True) + ea.sum(-1, keepdims=True)
    return (np.einsum("bhqk,bhkd->bhqd", (ep / den).astype(np.float32), V_prior)
            + np.einsum("bhqk,bhkd->bhqd", (ea / den).astype(np.float32),
                        V_active)).astype(np.float32)


def kernel(**inputs):
    Q = np.asarray(inputs["Q"], dtype=np.float32)
    K_prior = np.asarray(inputs["K_prior"], dtype=np.float32)
    V_prior = np.asarray(inputs["V_prior"], dtype=np.float32)
    K_active = np.asarray(inputs["K_active"], dtype=np.float32)
    V_active = np.asarray(inputs["V_active"], dtype=np.float32)
    prior_mask = np.asarray(inputs["prior_mask"])

    if not prior_mask.all():
        # Spec guarantees an all-ones mask; general masks take the slow path.
        return _numpy_fallback(Q, K_prior, V_prior, K_active, V_active,
                               prior_mask)

    nc = _get_compiled()
    res = run_bass_kernel_spmd(
        nc,
        make_in_maps(Q, K_prior, V_prior, K_active, V_active),
        core_ids=list(range(N_CORES)),
    )
    return gather_out([res.results[c]["out"] for c in range(N_CORES)])
